# revision 46
# baseline (speedup 1.0000x reference)
"""Trainium2 Bass kernel for BMeshRasterizer (pytorch3d-style naive rasterization).

Strategy:
  - Host: bin faces into 512 spatial bins of 128 pixels (8 rows x 16 cols);
    per bin build a [4, NP*MAXF] coefficient matrix so every per-(pixel,face)
    quantity is affine in the pixel features [1, px, py, px^2+py^2].
  - Device (SPMD over 8 cores, 32 bin-pairs/core, data-dependent per-pair
    plane widths): per bin one PE matmul pair evaluates all planes into
    PSUM; custom fused DVE ops compute perspective barycentrics, the
    validity-masked depth key and (negated) signed square distances;
    hardware max8/max_index picks the 8 nearest faces per pixel; the raw
    barycentric/distance planes are DMA-exported.
  - Host: gather payloads at the selected positions, remap local face
    indices to global ids, apply hit masks, reshape.
"""

import numpy as np

H = W = 256
F_FACES = 4096
K = 8
EPS = 1e-8
BINH, BINW = 8, 16            # 128 pixels per bin
NBX, NBY = W // BINW, H // BINH   # 16 x 32 = 512 bins
NBINS = NBX * NBY
NCORES = 8
BPC = NBINS // NCORES         # 64 bins per core (4 bin-rows)
NPAIR = BPC // 2
MAXF = 64                     # padded faces per bin (actual max ~59)
NPA = 8                       # bank-A planes: t0,t1,t2,up,den,tau0,tau1,tau2
NPB = 3                       # bank-B planes: aq0,aq1,aq2
NP = NPA + NPB
FLT_MAX = float(np.finfo(np.float32).max)

_prog_cache = {}
_dve_cache = {}


def _register_dve_ops():
    """Register the rasterizer's custom DVE ops in concourse's registry.
    Idempotent; shas are computed at registration so the pin always matches."""
    if _dve_cache:
        return _dve_cache
    from concourse import dve_ops
    from concourse.dve_spec import (
        Spec, Src0, Src1, C0, C2, Zero, One, MaxNeg,
        eq, ne, sq, select, maxx, minn, lower, _has_src1,
    )
    from concourse.dve_uop import DveOpSpec
    from operator import add

    fmax = np.float32(FLT_MAX)

    def ref_rsafe(in0, in1, c0, c1, c2):
        return np.where((in0 * in0 >= c2) | np.isnan(in0),
                        np.float32(0), in0).astype(np.float32)

    def ref_and2(in0, in1, c0, c1, c2):
        return np.where((in0 > 0) & (in1 > 0),
                        np.float32(1), np.float32(0)).astype(np.float32)

    def ref_pvsel(in0, in1, c0, c1, c2):
        return np.where((in0 > 0) & (in1 <= 0), in1, -fmax).astype(np.float32)

    def ref_znsel(in0, in1, c0, c1, c2):
        return np.where(in0 > 0, in1, -fmax).astype(np.float32)

    def ref_hseg(in0, in1, c0, c1, c2):
        c = np.clip(in0, 0.0, 1.0).astype(np.float32)
        return (c * (in0 + in0 - c) - in1).astype(np.float32)

    def ref_gatherv(in0, in1, c0, c1, c2):
        out = np.where(in1 == c0, in0, np.float32(0)).astype(np.float32)
        return out, out.reshape(out.shape[0], -1).sum(-1, keepdims=True)

    def ref_selmask(in0, in1, c0, c1, c2):
        return np.where((in1 >= c0) & (in1 > c2), in0, -fmax).astype(np.float32)

    _c = minn(maxx(Src0, Zero), One)
    defs = [
        ("RAST_RSAFE",
         Spec(body=select((sq(Src0) >= C2) | ne(Src0, Src0), Zero, Src0),
              reference=ref_rsafe)),
        ("RAST_AND2",
         Spec(body=select((Src0 > Zero) & (Src1 > Zero), One, Zero),
              reference=ref_and2)),
        ("RAST_PVSEL",
         Spec(body=select((Src0 > Zero) & (Src1 <= Zero), Src1, MaxNeg),
              reference=ref_pvsel)),
        ("RAST_ZNSEL",
         Spec(body=select(Src0 > Zero, Src1, MaxNeg),
              reference=ref_znsel)),
        ("RAST_HSEG",
         Spec(body=_c * (Src0 + Src0 - _c) - Src1,
              reference=ref_hseg)),
        ("RAST_GATHERV",
         Spec(body=select(eq(Src1, C0), Src0, Zero), accum=add,
              reference=ref_gatherv)),
        ("RAST_SELMASK",
         Spec(body=select((Src1 >= C0) & (Src1 > C2), Src0, MaxNeg),
              reference=ref_selmask)),
    ]
    for name, spec in defs:
        if name in dve_ops._SUB_OPCODE_FOR_NAME:
            _dve_cache[name] = next(o for o in dve_ops.OPS if o.name == name)
            continue
        row = dve_ops._CUSTOM_DVE_ROW_BASE + len(dve_ops.OPS)
        assert row < 0x20
        dve_ops._SUB_OPCODE_FOR_NAME[name] = row
        shas = {}
        for ver in ("v3", "v4"):
            s = DveOpSpec(name=name, opcode=row, uops=lower(spec, ver=ver),
                          rd1_en=_has_src1(spec))
            shas[ver] = s.sha(ver)
        op = dve_ops.DveOp(name, spec, False, shas)
        dve_ops.OPS.append(op)
        dve_ops.CUSTOM_DVE_SPECS[name] = spec
        _dve_cache[name] = op
    return _dve_cache


def _host_prep(face_verts):
    """Bin faces and build per-bin-pair coefficient matrices. Returns per-core
    input maps plus the facelist for index remapping."""
    fv = np.asarray(face_verts, dtype=np.float32)
    v = fv.astype(np.float64)
    v0, v1, v2 = v[:, 0], v[:, 1], v[:, 2]
    z0, z1, z2 = v0[:, 2], v1[:, 2], v2[:, 2]

    area = (v2[:, 0] - v0[:, 0]) * (v1[:, 1] - v0[:, 1]) - (
        v2[:, 1] - v0[:, 1]) * (v1[:, 0] - v0[:, 0])

    xmin = v[:, :, 0].min(1); xmax = v[:, :, 0].max(1)
    ymin = v[:, :, 1].min(1); ymax = v[:, :, 1].max(1)
    i_lo = np.floor((W - 1 - W * xmax) / 2).astype(int) - 1
    i_hi = np.ceil((W - 1 - W * xmin) / 2).astype(int) + 1
    j_lo = np.floor((H - 1 - H * ymax) / 2).astype(int) - 1
    j_hi = np.ceil((H - 1 - H * ymin) / 2).astype(int) + 1
    keep = (np.abs(area) > EPS) & (i_lo <= W - 1) & (i_hi >= 0) \
        & (j_lo <= H - 1) & (j_hi >= 0)

    bins = [[] for _ in range(NBINS)]
    for f in np.nonzero(keep)[0]:
        bx0 = max(i_lo[f], 0) // BINW; bx1 = min(i_hi[f], W - 1) // BINW
        by0 = max(j_lo[f], 0) // BINH; by1 = min(j_hi[f], H - 1) // BINH
        for by in range(by0, by1 + 1):
            for bx in range(bx0, bx1 + 1):
                bins[by * NBX + bx].append(f)
    cnt = np.array([len(b) for b in bins])
    maxf = int(cnt.max())
    if maxf > MAXF:
        raise RuntimeError(f"bin overflow: {maxf} > MAXF={MAXF}")

    facelist = np.zeros((NBINS, MAXF), dtype=np.int64)
    for b in range(NBINS):
        fl = bins[b]
        facelist[b, :len(fl)] = fl

    # per-face affine coefficients (f64, cast to f32 at the end)
    def edge_affine(ax, ay, bx, by):
        # edge(p; a,b) = px*(by-ay) - py*(bx-ax) + [ay*(bx-ax) - ax*(by-ay)]
        return by - ay, -(bx - ax), ay * (bx - ax) - ax * (by - ay)

    e0x, e0y, e0c = edge_affine(v1[:, 0], v1[:, 1], v2[:, 0], v2[:, 1])
    e1x, e1y, e1c = edge_affine(v2[:, 0], v2[:, 1], v0[:, 0], v0[:, 1])
    e2x, e2y, e2c = edge_affine(v0[:, 0], v0[:, 1], v1[:, 0], v1[:, 1])
    # area cancels in the perspective ratio, so fold only the z-products
    s0, s1, s2 = z1 * z2, z0 * z2, z0 * z1
    t0 = (e0x * s0, e0y * s0, e0c * s0)
    t1 = (e1x * s1, e1y * s1, e1c * s1)
    t2 = (e2x * s2, e2y * s2, e2c * s2)
    up = tuple(-(z0 * a + z1 * b + z2 * c) for a, b, c in zip(t0, t1, t2))
    dn = tuple(a + b + c for a, b, c in zip(t0, t1, t2))

    def seg_coef(a, b):
        abx = b[:, 0] - a[:, 0]; aby = b[:, 1] - a[:, 1]
        q = np.maximum(abx * abx + aby * aby, EPS)
        return (abx / q, aby / q, -(a[:, 0] * abx + a[:, 1] * aby) / q, q)

    g0 = seg_coef(v0, v1); g1 = seg_coef(v1, v2); g2 = seg_coef(v2, v0)

    def apsq_q_coef(a, q):
        # |p-a|^2 / q  = (1*pp - 2ax*px - 2ay*py + (ax^2+ay^2)) / q
        return (-2 * a[:, 0] / q, -2 * a[:, 1] / q,
                (a[:, 0] ** 2 + a[:, 1] ** 2) / q, 1.0 / q)

    a0 = apsq_q_coef(v0, g0[3]); a1 = apsq_q_coef(v1, g1[3])
    a2 = apsq_q_coef(v2, g2[3])

    # coef[f, 4, NP]: rows are features [1, px, py, pp]
    coef = np.zeros((F_FACES, 4, NP), dtype=np.float64)

    def put(i, cx, cy, cc, cp=None):
        coef[:, 0, i] = cc; coef[:, 1, i] = cx; coef[:, 2, i] = cy
        if cp is not None:
            coef[:, 3, i] = cp

    put(0, *t0); put(1, *t1); put(2, *t2); put(3, *up); put(4, *dn)
    put(5, g0[0], g0[1], g0[2]); put(6, g1[0], g1[1], g1[2])
    put(7, g2[0], g2[1], g2[2])
    put(8, a0[0], a0[1], a0[2], a0[3]); put(9, a1[0], a1[1], a1[2], a1[3])
    put(10, a2[0], a2[1], a2[2], a2[3])
    coef32 = coef.astype(np.float32)
    qs = np.stack([g0[3], g1[3], g2[3]], 1).astype(np.float32)  # [F, 3]

    # per-bin coefficient matrices [NBINS, 4, NP, MAXF], padded slots zero
    binco = np.zeros((NBINS, 4, NP, MAXF), dtype=np.float32)
    binq = np.zeros((NBINS, 3, MAXF), dtype=np.float32)
    for b in range(NBINS):
        fl = bins[b]
        if fl:
            binco[b, :, :, :len(fl)] = coef32[fl].transpose(1, 2, 0)
            binq[b, :, :len(fl)] = qs[fl].T

    # pixel features per bin [NBINS, 4, 128]
    xs = ((W - 1 - 2.0 * np.arange(W)) / W).astype(np.float32)
    ys = ((H - 1 - 2.0 * np.arange(H)) / H).astype(np.float32)
    feat = np.zeros((NBINS, 4, BINH * BINW), dtype=np.float32)
    for by in range(NBY):
        for bx in range(NBX):
            b = by * NBX + bx
            px = np.tile(xs[bx * BINW:(bx + 1) * BINW], BINH)
            py = np.repeat(ys[by * BINH:(by + 1) * BINH], BINW)
            feat[b, 0] = 1.0
            feat[b, 1] = px
            feat[b, 2] = py
            feat[b, 3] = (px * px + py * py).astype(np.float32)

    # order bins within each core by face count, pair adjacent so each
    # pair's compute width can shrink to its own max count (rounded to 8)
    perm = np.zeros((NCORES, BPC), dtype=np.int64)     # slot -> global bin id
    for c in range(NCORES):
        gb = np.arange(c * BPC, (c + 1) * BPC)
        order = np.argsort(cnt[gb], kind="stable")
        perm[c] = gb[order]
    pair_cnt = cnt[perm].reshape(NCORES, NPAIR, 2).max(-1)     # [8, NPAIR]
    widths = np.clip((pair_cnt.max(0) + 7) // 8 * 8, 8, MAXF).astype(int)

    GRP, QG = 4, 8
    in_maps = []
    for c in range(NCORES):
        pb_ = perm[c]
        bc = binco[pb_].reshape(NPAIR // GRP, GRP, 2, 4, NP, MAXF)
        bc = bc.transpose(0, 3, 1, 2, 4, 5)
        ft = feat[pb_].reshape(NPAIR // GRP, GRP, 2, 4, BINH * BINW)
        ft = ft.transpose(0, 3, 1, 2, 4)
        qb = np.broadcast_to(
            binq[pb_].reshape(NPAIR // QG, 1, QG, 2, 3, MAXF),
            (NPAIR // QG, 128, QG, 2, 3, MAXF))
        in_maps.append({
            "coef": np.ascontiguousarray(bc),
            "feat": np.ascontiguousarray(ft),
            "qb": np.ascontiguousarray(qb),
        })
    return in_maps, facelist, perm, tuple(widths.tolist())


def _build_program(widths):
    import concourse.bass as bass
    import concourse.tile as tile
    from concourse import bacc, mybir

    ops = _register_dve_ops()
    RSAFE = ops["RAST_RSAFE"]; AND2 = ops["RAST_AND2"]
    PVSEL = ops["RAST_PVSEL"]; ZNSEL = ops["RAST_ZNSEL"]
    HSEG = ops["RAST_HSEG"]; SELMASK = ops["RAST_SELMASK"]

    f32 = mybir.dt.float32
    u32 = mybir.dt.uint32
    op = mybir.AluOpType
    ACopy = mybir.ActivationFunctionType.Copy

    nc = bacc.Bacc("TRN2", target_bir_lowering=False, debug=False)

    GRP = 4                   # coef/feat DMA batch (pairs)
    QG = 8                    # qb DMA batch (pairs)
    coef_d = nc.dram_tensor("coef", [NPAIR // GRP, 4, GRP, 2, NP, MAXF], f32,
                            kind="ExternalInput")
    feat_d = nc.dram_tensor("feat", [NPAIR // GRP, 4, GRP, 2, BINH * BINW],
                            f32, kind="ExternalInput")
    qb_d = nc.dram_tensor("qb", [NPAIR // QG, 128, QG, 2, 3, MAXF], f32,
                          kind="ExternalInput")
    ov_d = nc.dram_tensor("ov", [128, BPC * K], f32, kind="ExternalOutput")
    oi_d = nc.dram_tensor("oi", [128, BPC * K], u32, kind="ExternalOutput")
    pay_d = [nc.dram_tensor(f"op{n}", [128, NPAIR, 2, MAXF], f32,
                            kind="ExternalOutput") for n in ("w0", "w1", "sd")]

    with tile.TileContext(nc) as tc:
        with (
            tc.tile_pool(name="inp", bufs=2) as inp,
            tc.tile_pool(name="psum", bufs=2, space=bass.MemorySpace.PSUM) as psp,
            tc.tile_pool(name="wk", bufs=4) as wk,
            tc.tile_pool(name="stage", bufs=1) as stp,
        ):
            st_v = stp.tile([128, BPC * K], f32, tag="st_v")
            st_i = stp.tile([128, BPC * K], u32, tag="st_i")
            st_p = [stp.tile([128, NPAIR, 2, MAXF], f32, name=f"st_p{i}",
                             tag=f"st_p{i}") for i in range(3)]
            for i in range(3):
                nc.gpsimd.memset(st_p[i][:], 0.0)

            ct4 = ft4 = qb8 = None
            for p_ in range(NPAIR):
                w = int(widths[p_])
                g, gi = divmod(p_, GRP)
                if gi == 0:
                    ct4 = inp.tile([4, GRP, 2, NP, MAXF], f32, tag="ct")
                    nc.sync.dma_start(ct4[:], coef_d[g])
                    ft4 = inp.tile([4, GRP, 2, BINH * BINW], f32, tag="ft")
                    nc.sync.dma_start(ft4[:], feat_d[g])
                qg, qgi = divmod(p_, QG)
                if qgi == 0:
                    qb8 = inp.tile([128, QG, 2, 3, MAXF], f32, tag="qb")
                    nc.sync.dma_start(qb8[:], qb_d[qg])
                ct = ct4[:, gi]
                ft = ft4[:, gi]
                qb = qb8[:, qgi]

                # flat width-packed PSUM: bin j at a bank boundary, plane i of
                # bin j at [j*BANK + i*w, ... + w) -- matmul outs contiguous
                BANK = 512
                pa = psp.tile([128, 2, BANK], f32, tag="pa")
                pb = psp.tile([128, 2, 256], f32, tag="pb")
                for j in range(2):
                    nc.tensor.matmul(pa[:, j, 0:NPA * w], ft[:, j],
                                     ct[:, j, 0:NPA, 0:w],
                                     start=True, stop=True)
                    nc.tensor.matmul(pb[:, j, 0:NPB * w], ft[:, j],
                                     ct[:, j, NPA:, 0:w],
                                     start=True, stop=True)

                def pav(i):
                    return pa[:, :, i * w:(i + 1) * w]

                def pbv(i):
                    return pb[:, :, i * w:(i + 1) * w]

                # stage all PSUM planes to SBUF on the (mostly idle) scalar
                # engine: DVE pays 120 init cycles per PSUM operand vs 58 SBUF
                sba = wk.tile([128, 2, NPA * MAXF], f32, tag="sba")
                sbb = wk.tile([128, 2, NPB * MAXF], f32, tag="sbb")
                for j in range(2):
                    nc.scalar.activation(sba[:, j, 0:NPA * w],
                                         pa[:, j, 0:NPA * w], ACopy)
                    nc.scalar.activation(sbb[:, j, 0:NPB * w],
                                         pb[:, j, 0:NPB * w], ACopy)

                def sav(i):
                    return sba[:, :, i * w:(i + 1) * w]

                def sbv(i):
                    return sbb[:, :, i * w:(i + 1) * w]

                r_ = wk.tile([128, 2, MAXF], f32, tag="r")
                nc.vector.reciprocal(r_[:, :, 0:w], sav(4))
                rs = wk.tile([128, 2, MAXF], f32, tag="rs")
                nc.vector._custom_dve(RSAFE, out=rs[:, :, 0:w],
                                      in0=r_[:, :, 0:w], imm2=1e36)

                w0 = st_p[0][:, p_]
                nc.vector.tensor_tensor(w0[:, :, 0:w], sav(0),
                                        rs[:, :, 0:w], op.mult)
                w1 = st_p[1][:, p_]
                nc.vector.tensor_tensor(w1[:, :, 0:w], sav(1),
                                        rs[:, :, 0:w], op.mult)
                w2 = wk.tile([128, 2, MAXF], f32, tag="w2")
                nc.vector.tensor_tensor(w2[:, :, 0:w], sav(2),
                                        rs[:, :, 0:w], op.mult)
                pzn = wk.tile([128, 2, MAXF], f32, tag="pzn")
                nc.vector.tensor_tensor(pzn[:, :, 0:w], sav(3),
                                        rs[:, :, 0:w], op.mult)

                m01 = wk.tile([128, 2, MAXF], f32, tag="m01")
                nc.vector._custom_dve(AND2, out=m01[:, :, 0:w],
                                      in0=w0[:, :, 0:w], in1=w1[:, :, 0:w])
                pv = wk.tile([128, 2, MAXF], f32, tag="pv")
                nc.vector._custom_dve(PVSEL, out=pv[:, :, 0:w],
                                      in0=w2[:, :, 0:w], in1=pzn[:, :, 0:w])
                zn = wk.tile([128, 2, MAXF], f32, tag="zn")
                nc.vector._custom_dve(ZNSEL, out=zn[:, :, 0:w],
                                      in0=m01[:, :, 0:w], in1=pv[:, :, 0:w])

                # sd = max_j q_j*(h(tau_j) - apsq_j/q_j)  (negated min sq dist)
                sd = None
                for js in range(3):
                    e = wk.tile([128, 2, MAXF], f32, tag="e")
                    nc.vector._custom_dve(HSEG, out=e[:, :, 0:w],
                                          in0=sav(5 + js),
                                          in1=sbv(js))
                    if js == 0:
                        sd = wk.tile([128, 2, MAXF], f32, tag="mq0")
                        nc.vector.tensor_tensor(sd[:, :, 0:w], e[:, :, 0:w],
                                                qb[:, :, js, 0:w], op.mult)
                        continue
                    m_ = wk.tile([128, 2, MAXF], f32, tag=f"mq{js}")
                    nc.vector.tensor_tensor(m_[:, :, 0:w], e[:, :, 0:w],
                                            qb[:, :, js, 0:w], op.mult)
                    if js == 1:
                        sd2 = wk.tile([128, 2, MAXF], f32, tag="sd2")
                        nc.vector.tensor_tensor(sd2[:, :, 0:w], sd[:, :, 0:w],
                                                m_[:, :, 0:w], op.max)
                        sd = sd2
                    else:
                        sdf = st_p[2][:, p_]
                        nc.vector.tensor_tensor(sdf[:, :, 0:w], sd[:, :, 0:w],
                                                m_[:, :, 0:w], op.max)

                for j in range(2):
                    b = 2 * p_ + j
                    znj = zn[:, j, 0:w]
                    sl8 = slice(b * K, (b + 1) * K)
                    vals = st_v[:, sl8]
                    nc.vector.max(vals, znj)
                    nc.vector.max_index(st_i[:, sl8], vals, znj)

            # export the raw payload planes once; the host gathers them at
            # the selected positions (DMA has slack, DVE is the wall)
            for pi in range(3):
                nc.sync.dma_start(pay_d[pi][:], st_p[pi][:])
            nc.sync.dma_start(ov_d[:], st_v[:])
            nc.sync.dma_start(oi_d[:], st_i[:])

    nc.compile()
    return nc


def _get_program(widths):
    if widths not in _prog_cache:
        _prog_cache[widths] = _build_program(widths)
    return _prog_cache[widths]


def run(inputs, profile=False):
    from concourse.bass_utils import run_bass_kernel_spmd

    face_verts = np.asarray(inputs["face_verts"], dtype=np.float32)
    image_size = int(np.asarray(inputs["image_size"]))
    faces_per_pixel = int(np.asarray(inputs["faces_per_pixel"]))
    assert image_size == H and faces_per_pixel == K, \
        f"kernel hardcoded for image_size={H}, K={K}"
    assert face_verts.shape == (F_FACES, 3, 3)

    in_maps, facelist, perm, widths = _host_prep(face_verts)
    nc = _get_program(widths)
    kw = {}
    if profile:
        kw = dict(trace=True)
    res = run_bass_kernel_spmd(nc, in_maps, list(range(NCORES)), **kw)

    out = _assemble(res.results, facelist, perm)
    if profile:
        return out, res
    return out


def _assemble(results, facelist, perm):
    res = type("R", (), {"results": results})

    def to_image(name, dt):
        # [8 cores][128 pix, 64 slots * K] -> [H, W, K] via slot->bin perm
        a = np.stack([res.results[c][name] for c in range(NCORES)])
        a = a.reshape(NCORES, 128, BPC, K)
        binarr = np.zeros((NBINS, 128, K), dtype=dt)
        for c in range(NCORES):
            binarr[perm[c]] = a[c].transpose(1, 0, 2).astype(dt)
        # bin (by,bx), pixel p=(r,cc) -> image[by*BINH+r, bx*BINW+cc]
        img = binarr.reshape(NBY, NBX, BINH, BINW, K)
        img = img.transpose(0, 2, 1, 3, 4).reshape(H, W, K)
        return img

    vals = to_image("ov", np.float32)
    lidx = to_image("oi", np.int64)

    # gather payloads from the exported raw planes at the selected positions
    pz_clip = np.clip(lidx, 0, MAXF - 1)                   # [H, W, K]
    pay = []
    for n in ("w0", "w1", "sd"):
        a = np.stack([res.results[c][f"op{n}"] for c in range(NCORES)])
        # [8, 128, NPAIR, 2, MAXF] -> slot-major [8, BPC, 128, MAXF]
        a = a.transpose(0, 2, 3, 1, 4).reshape(NCORES, BPC, 128, MAXF)
        binarr = np.zeros((NBINS, 128, MAXF), np.float32)
        for c in range(NCORES):
            binarr[perm[c]] = a[c]
        img = binarr.reshape(NBY, NBX, BINH, BINW, MAXF)
        img = img.transpose(0, 2, 1, 3, 4).reshape(H, W, MAXF)
        pay.append(np.take_along_axis(img, pz_clip, axis=-1))
    gw0, gw1, gsd = pay

    hit = vals > -5e29
    zbuf = np.where(hit, -vals, np.float32(-1.0)).astype(np.float32)

    # remap local face index -> global face id via per-bin facelists
    bx_of = np.tile(np.repeat(np.arange(NBX), BINW), H).reshape(H, W)
    by_of = np.repeat(np.arange(NBY), BINH)[:, None].repeat(W, 1)
    bin_of = (by_of * NBX + bx_of)[..., None]              # [H,W,1]
    gidx = facelist[bin_of, np.clip(lidx, 0, MAXF - 1)]
    p2f = np.where(hit, gidx, -1).astype(np.int32)

    w0 = np.where(hit, gw0, np.float32(-1.0)).astype(np.float32)
    w1 = np.where(hit, gw1, np.float32(-1.0)).astype(np.float32)
    w2 = np.where(hit, np.float32(1.0) - w0 - w1,
                  np.float32(-1.0)).astype(np.float32)
    bary = np.stack([w0, w1, w2], axis=-1)
    dists = np.where(hit, gsd, np.float32(-1.0)).astype(np.float32)

    return (p2f[None], zbuf[None], bary[None], dists[None])


def kernel(**inputs):
    return run(inputs)


# revision 48
# speedup vs baseline: 1.0165x; 1.0165x over previous
"""Trainium2 Bass kernel for BMeshRasterizer (pytorch3d-style naive rasterization).

Strategy:
  - Host: bin faces into 512 spatial bins of 128 pixels (8 rows x 16 cols);
    per bin build a [4, NP*MAXF] coefficient matrix so every per-(pixel,face)
    quantity is affine in the pixel features [1, px, py, px^2+py^2].
  - Device (SPMD over 8 cores, 32 bin-pairs/core, data-dependent per-pair
    plane widths): per bin one PE matmul pair evaluates all planes into
    PSUM; custom fused DVE ops compute perspective barycentrics, the
    validity-masked depth key and (negated) signed square distances;
    hardware max8/max_index picks the 8 nearest faces per pixel; the raw
    barycentric/distance planes are DMA-exported.
  - Host: gather payloads at the selected positions, remap local face
    indices to global ids, apply hit masks, reshape.
"""

import numpy as np

H = W = 256
F_FACES = 4096
K = 8
EPS = 1e-8
BINH, BINW = 8, 16            # 128 pixels per bin
NBX, NBY = W // BINW, H // BINH   # 16 x 32 = 512 bins
NBINS = NBX * NBY
NCORES = 8
BPC = NBINS // NCORES         # 64 bins per core (4 bin-rows)
NPAIR = BPC // 2
MAXF = 64                     # padded faces per bin (actual max ~59)
NPA = 8                       # bank-A planes: t0,t1,t2,up,den,tau0,tau1,tau2
NPB = 3                       # bank-B planes: aq0,aq1,aq2
NP = NPA + NPB
FLT_MAX = float(np.finfo(np.float32).max)

_prog_cache = {}
_dve_cache = {}


def _register_dve_ops():
    """Register the rasterizer's custom DVE ops in concourse's registry.
    Idempotent; shas are computed at registration so the pin always matches."""
    if _dve_cache:
        return _dve_cache
    from concourse import dve_ops
    from concourse.dve_spec import (
        Spec, Src0, Src1, C0, C2, Zero, One, MaxNeg,
        eq, ne, sq, select, maxx, minn, lower, _has_src1,
    )
    from concourse.dve_uop import DveOpSpec
    from operator import add

    fmax = np.float32(FLT_MAX)

    def ref_rsafe(in0, in1, c0, c1, c2):
        return np.where((in0 * in0 >= c2) | np.isnan(in0),
                        np.float32(0), in0).astype(np.float32)

    def ref_and2(in0, in1, c0, c1, c2):
        return np.where((in0 > 0) & (in1 > 0),
                        np.float32(1), np.float32(0)).astype(np.float32)

    def ref_pvsel(in0, in1, c0, c1, c2):
        return np.where((in0 > 0) & (in1 <= 0), in1, -fmax).astype(np.float32)

    def ref_znsel(in0, in1, c0, c1, c2):
        return np.where(in0 > 0, in1, -fmax).astype(np.float32)

    def ref_hseg(in0, in1, c0, c1, c2):
        c = np.clip(in0, 0.0, 1.0).astype(np.float32)
        return (c * (in0 + in0 - c) - in1).astype(np.float32)

    def ref_gatherv(in0, in1, c0, c1, c2):
        out = np.where(in1 == c0, in0, np.float32(0)).astype(np.float32)
        return out, out.reshape(out.shape[0], -1).sum(-1, keepdims=True)

    def ref_selmask(in0, in1, c0, c1, c2):
        return np.where((in1 >= c0) & (in1 > c2), in0, -fmax).astype(np.float32)

    _c = minn(maxx(Src0, Zero), One)
    defs = [
        ("RAST_RSAFE",
         Spec(body=select((sq(Src0) >= C2) | ne(Src0, Src0), Zero, Src0),
              reference=ref_rsafe)),
        ("RAST_AND2",
         Spec(body=select((Src0 > Zero) & (Src1 > Zero), One, Zero),
              reference=ref_and2)),
        ("RAST_PVSEL",
         Spec(body=select((Src0 > Zero) & (Src1 <= Zero), Src1, MaxNeg),
              reference=ref_pvsel)),
        ("RAST_ZNSEL",
         Spec(body=select(Src0 > Zero, Src1, MaxNeg),
              reference=ref_znsel)),
        ("RAST_HSEG",
         Spec(body=_c * (Src0 + Src0 - _c) - Src1,
              reference=ref_hseg)),
        ("RAST_GATHERV",
         Spec(body=select(eq(Src1, C0), Src0, Zero), accum=add,
              reference=ref_gatherv)),
        ("RAST_SELMASK",
         Spec(body=select((Src1 >= C0) & (Src1 > C2), Src0, MaxNeg),
              reference=ref_selmask)),
    ]
    for name, spec in defs:
        if name in dve_ops._SUB_OPCODE_FOR_NAME:
            _dve_cache[name] = next(o for o in dve_ops.OPS if o.name == name)
            continue
        row = dve_ops._CUSTOM_DVE_ROW_BASE + len(dve_ops.OPS)
        assert row < 0x20
        dve_ops._SUB_OPCODE_FOR_NAME[name] = row
        shas = {}
        for ver in ("v3", "v4"):
            s = DveOpSpec(name=name, opcode=row, uops=lower(spec, ver=ver),
                          rd1_en=_has_src1(spec))
            shas[ver] = s.sha(ver)
        op = dve_ops.DveOp(name, spec, False, shas)
        dve_ops.OPS.append(op)
        dve_ops.CUSTOM_DVE_SPECS[name] = spec
        _dve_cache[name] = op
    return _dve_cache


def _host_prep(face_verts):
    """Bin faces and build per-bin-pair coefficient matrices. Returns per-core
    input maps plus the facelist for index remapping."""
    fv = np.asarray(face_verts, dtype=np.float32)
    v = fv.astype(np.float64)
    v0, v1, v2 = v[:, 0], v[:, 1], v[:, 2]
    z0, z1, z2 = v0[:, 2], v1[:, 2], v2[:, 2]

    area = (v2[:, 0] - v0[:, 0]) * (v1[:, 1] - v0[:, 1]) - (
        v2[:, 1] - v0[:, 1]) * (v1[:, 0] - v0[:, 0])

    xmin = v[:, :, 0].min(1); xmax = v[:, :, 0].max(1)
    ymin = v[:, :, 1].min(1); ymax = v[:, :, 1].max(1)
    i_lo = np.floor((W - 1 - W * xmax) / 2).astype(int) - 1
    i_hi = np.ceil((W - 1 - W * xmin) / 2).astype(int) + 1
    j_lo = np.floor((H - 1 - H * ymax) / 2).astype(int) - 1
    j_hi = np.ceil((H - 1 - H * ymin) / 2).astype(int) + 1
    keep = (np.abs(area) > EPS) & (i_lo <= W - 1) & (i_hi >= 0) \
        & (j_lo <= H - 1) & (j_hi >= 0)

    bins = [[] for _ in range(NBINS)]
    for f in np.nonzero(keep)[0]:
        bx0 = max(i_lo[f], 0) // BINW; bx1 = min(i_hi[f], W - 1) // BINW
        by0 = max(j_lo[f], 0) // BINH; by1 = min(j_hi[f], H - 1) // BINH
        for by in range(by0, by1 + 1):
            for bx in range(bx0, bx1 + 1):
                bins[by * NBX + bx].append(f)
    cnt = np.array([len(b) for b in bins])
    maxf = int(cnt.max())
    if maxf > MAXF:
        raise RuntimeError(f"bin overflow: {maxf} > MAXF={MAXF}")

    facelist = np.zeros((NBINS, MAXF), dtype=np.int64)
    for b in range(NBINS):
        fl = bins[b]
        facelist[b, :len(fl)] = fl

    # per-face affine coefficients (f64, cast to f32 at the end)
    def edge_affine(ax, ay, bx, by):
        # edge(p; a,b) = px*(by-ay) - py*(bx-ax) + [ay*(bx-ax) - ax*(by-ay)]
        return by - ay, -(bx - ax), ay * (bx - ax) - ax * (by - ay)

    e0x, e0y, e0c = edge_affine(v1[:, 0], v1[:, 1], v2[:, 0], v2[:, 1])
    e1x, e1y, e1c = edge_affine(v2[:, 0], v2[:, 1], v0[:, 0], v0[:, 1])
    e2x, e2y, e2c = edge_affine(v0[:, 0], v0[:, 1], v1[:, 0], v1[:, 1])
    # area cancels in the perspective ratio, so fold only the z-products
    s0, s1, s2 = z1 * z2, z0 * z2, z0 * z1
    t0 = (e0x * s0, e0y * s0, e0c * s0)
    t1 = (e1x * s1, e1y * s1, e1c * s1)
    t2 = (e2x * s2, e2y * s2, e2c * s2)
    up = tuple(-(z0 * a + z1 * b + z2 * c) for a, b, c in zip(t0, t1, t2))
    dn = tuple(a + b + c for a, b, c in zip(t0, t1, t2))

    def seg_coef(a, b):
        abx = b[:, 0] - a[:, 0]; aby = b[:, 1] - a[:, 1]
        q = np.maximum(abx * abx + aby * aby, EPS)
        return (abx / q, aby / q, -(a[:, 0] * abx + a[:, 1] * aby) / q, q)

    g0 = seg_coef(v0, v1); g1 = seg_coef(v1, v2); g2 = seg_coef(v2, v0)

    def apsq_q_coef(a, q):
        # |p-a|^2 / q  = (1*pp - 2ax*px - 2ay*py + (ax^2+ay^2)) / q
        return (-2 * a[:, 0] / q, -2 * a[:, 1] / q,
                (a[:, 0] ** 2 + a[:, 1] ** 2) / q, 1.0 / q)

    a0 = apsq_q_coef(v0, g0[3]); a1 = apsq_q_coef(v1, g1[3])
    a2 = apsq_q_coef(v2, g2[3])

    # coef[f, 4, NP]: rows are features [1, px, py, pp]
    coef = np.zeros((F_FACES, 4, NP), dtype=np.float64)

    def put(i, cx, cy, cc, cp=None):
        coef[:, 0, i] = cc; coef[:, 1, i] = cx; coef[:, 2, i] = cy
        if cp is not None:
            coef[:, 3, i] = cp

    put(0, *t0); put(1, *t1); put(2, *t2); put(3, *up); put(4, *dn)
    put(5, g0[0], g0[1], g0[2]); put(6, g1[0], g1[1], g1[2])
    put(7, g2[0], g2[1], g2[2])
    put(8, a0[0], a0[1], a0[2], a0[3]); put(9, a1[0], a1[1], a1[2], a1[3])
    put(10, a2[0], a2[1], a2[2], a2[3])
    coef32 = coef.astype(np.float32)
    qs = np.stack([g0[3], g1[3], g2[3]], 1).astype(np.float32)  # [F, 3]

    # per-bin coefficient matrices [NBINS, 4, NP, MAXF], padded slots zero
    binco = np.zeros((NBINS, 4, NP, MAXF), dtype=np.float32)
    binq = np.zeros((NBINS, 3, MAXF), dtype=np.float32)
    for b in range(NBINS):
        fl = bins[b]
        if fl:
            binco[b, :, :, :len(fl)] = coef32[fl].transpose(1, 2, 0)
            binq[b, :, :len(fl)] = qs[fl].T

    # pixel features per bin [NBINS, 4, 128]
    xs = ((W - 1 - 2.0 * np.arange(W)) / W).astype(np.float32)
    ys = ((H - 1 - 2.0 * np.arange(H)) / H).astype(np.float32)
    feat = np.zeros((NBINS, 4, BINH * BINW), dtype=np.float32)
    for by in range(NBY):
        for bx in range(NBX):
            b = by * NBX + bx
            px = np.tile(xs[bx * BINW:(bx + 1) * BINW], BINH)
            py = np.repeat(ys[by * BINH:(by + 1) * BINH], BINW)
            feat[b, 0] = 1.0
            feat[b, 1] = px
            feat[b, 2] = py
            feat[b, 3] = (px * px + py * py).astype(np.float32)

    # order bins within each core by face count, pair adjacent so each
    # pair's compute width can shrink to its own max count (rounded to 8)
    perm = np.zeros((NCORES, BPC), dtype=np.int64)     # slot -> global bin id
    for c in range(NCORES):
        gb = np.arange(c * BPC, (c + 1) * BPC)
        order = np.argsort(cnt[gb], kind="stable")
        perm[c] = gb[order]
    pair_cnt = cnt[perm].reshape(NCORES, NPAIR, 2).max(-1)     # [8, NPAIR]
    widths = np.clip((pair_cnt.max(0) + 7) // 8 * 8, 8, MAXF).astype(int)

    GRP, QG = 4, 8
    in_maps = []
    for c in range(NCORES):
        pb_ = perm[c]
        bc = binco[pb_].reshape(NPAIR // GRP, GRP, 2, 4, NP, MAXF)
        bc = bc.transpose(0, 3, 1, 2, 4, 5)
        ft = feat[pb_].reshape(NPAIR // GRP, GRP, 2, 4, BINH * BINW)
        ft = ft.transpose(0, 3, 1, 2, 4)
        qb = np.broadcast_to(
            binq[pb_].reshape(NPAIR // QG, 1, QG, 2, 3, MAXF),
            (NPAIR // QG, 128, QG, 2, 3, MAXF))
        in_maps.append({
            "coef": np.ascontiguousarray(bc),
            "feat": np.ascontiguousarray(ft),
            "qb": np.ascontiguousarray(qb),
        })
    return in_maps, facelist, perm, tuple(widths.tolist())


def _build_program(widths):
    import concourse.bass as bass
    import concourse.tile as tile
    from concourse import bacc, mybir

    ops = _register_dve_ops()
    RSAFE = ops["RAST_RSAFE"]; AND2 = ops["RAST_AND2"]
    PVSEL = ops["RAST_PVSEL"]; ZNSEL = ops["RAST_ZNSEL"]
    HSEG = ops["RAST_HSEG"]; SELMASK = ops["RAST_SELMASK"]

    f32 = mybir.dt.float32
    u32 = mybir.dt.uint32
    op = mybir.AluOpType
    ACopy = mybir.ActivationFunctionType.Copy

    nc = bacc.Bacc("TRN2", target_bir_lowering=False, debug=False)

    GRP = 4                   # coef/feat DMA batch (pairs)
    QG = 8                    # qb DMA batch (pairs)
    coef_d = nc.dram_tensor("coef", [NPAIR // GRP, 4, GRP, 2, NP, MAXF], f32,
                            kind="ExternalInput")
    feat_d = nc.dram_tensor("feat", [NPAIR // GRP, 4, GRP, 2, BINH * BINW],
                            f32, kind="ExternalInput")
    qb_d = nc.dram_tensor("qb", [NPAIR // QG, 128, QG, 2, 3, MAXF], f32,
                          kind="ExternalInput")
    ov_d = nc.dram_tensor("ov", [128, BPC * K], f32, kind="ExternalOutput")
    oi_d = nc.dram_tensor("oi", [128, BPC * K], u32, kind="ExternalOutput")
    pay_d = [nc.dram_tensor(f"op{n}", [128, NPAIR, 2, MAXF], f32,
                            kind="ExternalOutput") for n in ("w0", "w1", "sd")]

    with tile.TileContext(nc) as tc:
        with (
            tc.tile_pool(name="inp", bufs=2) as inp,
            tc.tile_pool(name="psum", bufs=2, space=bass.MemorySpace.PSUM) as psp,
            tc.tile_pool(name="wk", bufs=4) as wk,
            tc.tile_pool(name="stage", bufs=1) as stp,
        ):
            st_v = stp.tile([128, BPC * K], f32, tag="st_v")
            st_i = stp.tile([128, BPC * K], u32, tag="st_i")
            st_p = [stp.tile([128, NPAIR, 2, MAXF], f32, name=f"st_p{i}",
                             tag=f"st_p{i}") for i in range(3)]
            for i in range(3):
                nc.gpsimd.memset(st_p[i][:], 0.0)

            ct4 = ft4 = qb8 = None
            for p_ in range(NPAIR):
                w = int(widths[p_])
                g, gi = divmod(p_, GRP)
                if gi == 0:
                    ct4 = inp.tile([4, GRP, 2, NP, MAXF], f32, tag="ct")
                    nc.sync.dma_start(ct4[:], coef_d[g])
                    ft4 = inp.tile([4, GRP, 2, BINH * BINW], f32, tag="ft")
                    nc.sync.dma_start(ft4[:], feat_d[g])
                qg, qgi = divmod(p_, QG)
                if qgi == 0:
                    qb8 = inp.tile([128, QG, 2, 3, MAXF], f32, tag="qb")
                    nc.sync.dma_start(qb8[:], qb_d[qg])
                ct = ct4[:, gi]
                ft = ft4[:, gi]
                qb = qb8[:, qgi]

                # flat width-packed PSUM: bin j at a bank boundary, plane i of
                # bin j at [j*BANK + i*w, ... + w) -- matmul outs contiguous
                BANK = 512
                pa = psp.tile([128, 2, BANK], f32, tag="pa")
                pb = psp.tile([128, 2, 256], f32, tag="pb")
                for j in range(2):
                    nc.tensor.matmul(pa[:, j, 0:NPA * w], ft[:, j],
                                     ct[:, j, 0:NPA, 0:w],
                                     start=True, stop=True)
                    nc.tensor.matmul(pb[:, j, 0:NPB * w], ft[:, j],
                                     ct[:, j, NPA:, 0:w],
                                     start=True, stop=True)

                def pav(i):
                    return pa[:, :, i * w:(i + 1) * w]

                def pbv(i):
                    return pb[:, :, i * w:(i + 1) * w]

                # copy tau planes PSUM -> SBUF on the (idle) scalar engine so
                # the fused seg op can pair them with the PSUM aq planes
                tau = wk.tile([128, 2, 3 * MAXF], f32, tag="tau")
                for j in range(2):
                    nc.scalar.activation(tau[:, j, 0:3 * w],
                                         pa[:, j, 5 * w:8 * w], ACopy)

                r_ = wk.tile([128, 2, MAXF], f32, tag="r")
                nc.vector.reciprocal(r_[:, :, 0:w], pav(4))
                rs = wk.tile([128, 2, MAXF], f32, tag="rs")
                nc.vector._custom_dve(RSAFE, out=rs[:, :, 0:w],
                                      in0=r_[:, :, 0:w], imm2=1e36)

                w0 = st_p[0][:, p_]
                nc.vector.tensor_tensor(w0[:, :, 0:w], pav(0),
                                        rs[:, :, 0:w], op.mult)
                w1 = st_p[1][:, p_]
                nc.vector.tensor_tensor(w1[:, :, 0:w], pav(1),
                                        rs[:, :, 0:w], op.mult)
                w2 = wk.tile([128, 2, MAXF], f32, tag="w2")
                nc.vector.tensor_tensor(w2[:, :, 0:w], pav(2),
                                        rs[:, :, 0:w], op.mult)
                pzn = wk.tile([128, 2, MAXF], f32, tag="pzn")
                nc.vector.tensor_tensor(pzn[:, :, 0:w], pav(3),
                                        rs[:, :, 0:w], op.mult)

                m01 = wk.tile([128, 2, MAXF], f32, tag="m01")
                nc.vector._custom_dve(AND2, out=m01[:, :, 0:w],
                                      in0=w0[:, :, 0:w], in1=w1[:, :, 0:w])
                pv = wk.tile([128, 2, MAXF], f32, tag="pv")
                nc.vector._custom_dve(PVSEL, out=pv[:, :, 0:w],
                                      in0=w2[:, :, 0:w], in1=pzn[:, :, 0:w])
                zn = wk.tile([128, 2, MAXF], f32, tag="zn")
                nc.vector._custom_dve(ZNSEL, out=zn[:, :, 0:w],
                                      in0=m01[:, :, 0:w], in1=pv[:, :, 0:w])

                # sd = max_j q_j*(h(tau_j) - apsq_j/q_j)  (negated min sq dist)
                sd = None
                for js in range(3):
                    e = wk.tile([128, 2, MAXF], f32, tag="e")
                    nc.vector._custom_dve(HSEG, out=e[:, :, 0:w],
                                          in0=tau[:, :, js * w:(js + 1) * w],
                                          in1=pbv(js))
                    if js == 0:
                        sd = wk.tile([128, 2, MAXF], f32, tag="mq0")
                        nc.vector.tensor_tensor(sd[:, :, 0:w], e[:, :, 0:w],
                                                qb[:, :, js, 0:w], op.mult)
                        continue
                    m_ = wk.tile([128, 2, MAXF], f32, tag=f"mq{js}")
                    nc.vector.tensor_tensor(m_[:, :, 0:w], e[:, :, 0:w],
                                            qb[:, :, js, 0:w], op.mult)
                    if js == 1:
                        sd2 = wk.tile([128, 2, MAXF], f32, tag="sd2")
                        nc.vector.tensor_tensor(sd2[:, :, 0:w], sd[:, :, 0:w],
                                                m_[:, :, 0:w], op.max)
                        sd = sd2
                    else:
                        sdf = st_p[2][:, p_]
                        nc.vector.tensor_tensor(sdf[:, :, 0:w], sd[:, :, 0:w],
                                                m_[:, :, 0:w], op.max)

                for j in range(2):
                    b = 2 * p_ + j
                    znj = zn[:, j, 0:w]
                    sl8 = slice(b * K, (b + 1) * K)
                    vals = st_v[:, sl8]
                    nc.vector.max(vals, znj)
                    nc.vector.max_index(st_i[:, sl8], vals, znj)

            # export the raw payload planes once; the host gathers them at
            # the selected positions (DMA has slack, DVE is the wall)
            for pi in range(3):
                nc.sync.dma_start(pay_d[pi][:], st_p[pi][:])
            nc.sync.dma_start(ov_d[:], st_v[:])
            nc.sync.dma_start(oi_d[:], st_i[:])

    nc.compile()
    return nc


def _get_program(widths):
    if widths not in _prog_cache:
        _prog_cache[widths] = _build_program(widths)
    return _prog_cache[widths]


def run(inputs, profile=False):
    from concourse.bass_utils import run_bass_kernel_spmd

    face_verts = np.asarray(inputs["face_verts"], dtype=np.float32)
    image_size = int(np.asarray(inputs["image_size"]))
    faces_per_pixel = int(np.asarray(inputs["faces_per_pixel"]))
    assert image_size == H and faces_per_pixel == K, \
        f"kernel hardcoded for image_size={H}, K={K}"
    assert face_verts.shape == (F_FACES, 3, 3)

    in_maps, facelist, perm, widths = _host_prep(face_verts)
    nc = _get_program(widths)
    kw = {}
    if profile:
        kw = dict(trace=True)
    res = run_bass_kernel_spmd(nc, in_maps, list(range(NCORES)), **kw)

    out = _assemble(res.results, facelist, perm)
    if profile:
        return out, res
    return out


def _assemble(results, facelist, perm):
    res = type("R", (), {"results": results})

    def to_image(name, dt):
        # [8 cores][128 pix, 64 slots * K] -> [H, W, K] via slot->bin perm
        a = np.stack([res.results[c][name] for c in range(NCORES)])
        a = a.reshape(NCORES, 128, BPC, K)
        binarr = np.zeros((NBINS, 128, K), dtype=dt)
        for c in range(NCORES):
            binarr[perm[c]] = a[c].transpose(1, 0, 2).astype(dt)
        # bin (by,bx), pixel p=(r,cc) -> image[by*BINH+r, bx*BINW+cc]
        img = binarr.reshape(NBY, NBX, BINH, BINW, K)
        img = img.transpose(0, 2, 1, 3, 4).reshape(H, W, K)
        return img

    vals = to_image("ov", np.float32)
    lidx = to_image("oi", np.int64)

    # gather payloads from the exported raw planes at the selected positions
    pz_clip = np.clip(lidx, 0, MAXF - 1)                   # [H, W, K]
    pay = []
    for n in ("w0", "w1", "sd"):
        a = np.stack([res.results[c][f"op{n}"] for c in range(NCORES)])
        # [8, 128, NPAIR, 2, MAXF] -> slot-major [8, BPC, 128, MAXF]
        a = a.transpose(0, 2, 3, 1, 4).reshape(NCORES, BPC, 128, MAXF)
        binarr = np.zeros((NBINS, 128, MAXF), np.float32)
        for c in range(NCORES):
            binarr[perm[c]] = a[c]
        img = binarr.reshape(NBY, NBX, BINH, BINW, MAXF)
        img = img.transpose(0, 2, 1, 3, 4).reshape(H, W, MAXF)
        pay.append(np.take_along_axis(img, pz_clip, axis=-1))
    gw0, gw1, gsd = pay

    hit = vals > -5e29
    zbuf = np.where(hit, -vals, np.float32(-1.0)).astype(np.float32)

    # remap local face index -> global face id via per-bin facelists
    bx_of = np.tile(np.repeat(np.arange(NBX), BINW), H).reshape(H, W)
    by_of = np.repeat(np.arange(NBY), BINH)[:, None].repeat(W, 1)
    bin_of = (by_of * NBX + bx_of)[..., None]              # [H,W,1]
    gidx = facelist[bin_of, np.clip(lidx, 0, MAXF - 1)]
    p2f = np.where(hit, gidx, -1).astype(np.int32)

    w0 = np.where(hit, gw0, np.float32(-1.0)).astype(np.float32)
    w1 = np.where(hit, gw1, np.float32(-1.0)).astype(np.float32)
    w2 = np.where(hit, np.float32(1.0) - w0 - w1,
                  np.float32(-1.0)).astype(np.float32)
    bary = np.stack([w0, w1, w2], axis=-1)
    dists = np.where(hit, gsd, np.float32(-1.0)).astype(np.float32)

    return (p2f[None], zbuf[None], bary[None], dists[None])


def kernel(**inputs):
    return run(inputs)
